# revision 1
# baseline (speedup 1.0000x reference)
"""DeltaLM Trainium2 Bass kernel.

8 NeuronCores = (batch 0..3) x (time-half 0..1), 1024 time rows per core.
Pair AllGather per layer carries the 128-row halo + running-sum vector.
All matmul paths run in fp32r (operands RNE-rounded to 11 mantissa bits,
exact fp32 accumulation); gather/scan/LayerNorm statistics stay exact fp32.
kernel(**inputs) -> np.ndarray (B, T, VOCAB) float32.
"""

import numpy as np
import concourse.bass as bass
import concourse.bacc as bacc
import concourse.mybir as mybir
from concourse.tile import TileContext
from concourse.bass import IndirectOffsetOnAxis

F32 = mybir.dt.float32
F32R = mybir.dt.float32r
U32 = mybir.dt.uint32
AX = mybir.AxisListType
OP = mybir.AluOpType
ACT = mybir.ActivationFunctionType

B, T, D, L, VOCAB = 4, 2048, 512, 3, 256
R = 1024          # rows per core
NB = 8            # 128-row blocks per core
H = 2048          # hidden
NHB = 16          # hidden blocks
ND = 4            # feature blocks (D/128)
CHUNK = 256       # mm1 row-chunk
NCH = R // CHUNK  # 4 chunks
RB_PER_CH = CHUNK // 128  # 2


def rne12(x):
    """Round fp32 array to fp32r (11 mantissa bits, RNE ties-even)."""
    u = x.astype(np.float32).view(np.uint32).astype(np.uint64)
    low = u & np.uint64(0xFFF)
    up = (u >> np.uint64(12)) & np.uint64(1)
    add = (low > 0x800) | ((low == 0x800) & (up == 1))
    u2 = (u & np.uint64(0xFFFFF000)) + np.uint64(0x1000) * add
    return u2.astype(np.uint32).view(np.float32)


def softplus64(x):
    x = np.asarray(x, np.float64)
    return np.log1p(np.exp(-np.abs(x))) + np.maximum(x, 0.0)


DEFAULT_CFG = dict(mlp="f32r", ctx="f32r", out="f32r", bd="f32r", w1_bufs=16, w2_bufs=16,
                   ps_mm_bufs=4, gt_bufs=2, repeat=1, no_cc=False, hw_loop=1)


def build(cfg=None):
    cfg = {**DEFAULT_CFG, **(cfg or {})}
    CTX32 = cfg["ctx"] == "f32"
    OUT32 = cfg["out"] == "f32"
    MLPDT = F32 if cfg["mlp"] == "f32" else F32R

    nc = bacc.Bacc("TRN2", target_bir_lowering=False, debug=False, num_devices=8)

    # ---- DRAM I/O (per-core shapes) ----
    import hashlib, json
    nonce_k = int(hashlib.sha256(json.dumps(cfg, sort_keys=True).encode()).hexdigest(), 16) % 97 + 2
    nonce_d = nc.dram_tensor("nonce", [128, nonce_k], F32, kind="ExternalInput")
    emb_d = nc.dram_tensor("emb", [VOCAB, D], F32, kind="ExternalInput")
    idx_d = nc.dram_tensor("idx", [128, NB], U32, kind="ExternalInput")
    sel_d = nc.dram_tensor("sel", [128, 1], F32, kind="ExternalInput")
    w1_d = nc.dram_tensor("w1", [L, D, H], MLPDT, kind="ExternalInput")
    w2_d = nc.dram_tensor("w2", [L, H, D], MLPDT, kind="ExternalInput")
    b1_d = nc.dram_tensor("b1", [L, 128, NHB], F32, kind="ExternalInput")
    b2_d = nc.dram_tensor("b2", [L, 1, D], MLPDT, kind="ExternalInput")
    ut32_d = nc.dram_tensor("ut32", [128, 128], F32, kind="ExternalInput")
    utr_d = nc.dram_tensor("utr", [128, 128], F32R, kind="ExternalInput")
    utms_d = nc.dram_tensor("utms", [128, 128], F32R, kind="ExternalInput")
    eneg_d = nc.dram_tensor("eneg", [128, 128], F32R, kind="ExternalInput")
    CDT = F32 if CTX32 else F32R
    dcyd_d = nc.dram_tensor("dcyd", [L, 128, 128], CDT, kind="ExternalInput")
    dcys_d = nc.dram_tensor("dcys", [L, 128, 128], CDT, kind="ExternalInput")
    ODT = F32 if OUT32 else F32R
    embT_d = nc.dram_tensor("embT", [D, VOCAB], ODT, kind="ExternalInput")
    idm_r_d = nc.dram_tensor("idm_r", [128, 128], F32R, kind="ExternalInput")
    idm32_d = nc.dram_tensor("idm32", [128, 128], F32, kind="ExternalInput")
    or32_d = nc.dram_tensor("or32", [1, 128], F32, kind="ExternalInput")
    orr_d = nc.dram_tensor("orr", [1, 128], F32R, kind="ExternalInput")
    oc32_d = nc.dram_tensor("oc32", [128, 1], F32, kind="ExternalInput")
    ocr_d = nc.dram_tensor("ocr", [128, 1], F32R, kind="ExternalInput")
    out_d = nc.dram_tensor("logitsT", [VOCAB, R], F32, kind="ExternalOutput")

    # collective buffers (plain local DRAM)
    cc_tc_in = nc.dram_tensor("cc_tc_in", [1, D], F32)
    cc_tc_out = nc.dram_tensor("cc_tc_out", [2, D], F32)
    cc_in = [nc.dram_tensor(f"cc_in{l}", [129, D], F32) for l in range(L)]
    cc_out = [nc.dram_tensor(f"cc_out{l}", [258, D], F32) for l in range(L)]

    GROUPS = [[0, 1], [2, 3], [4, 5], [6, 7]]

    with TileContext(nc) as tc:
        with (
            tc.tile_pool(name="const", bufs=1) as cst,
            tc.tile_pool(name="state", bufs=1) as st,
            tc.tile_pool(name="wts", bufs=1) as wp,
            tc.tile_pool(name="work", bufs=2) as wk,
            tc.tile_pool(name="rows", bufs=2) as rw,
            tc.tile_pool(name="psmm", bufs=cfg["ps_mm_bufs"], space="PSUM") as psm,
            tc.tile_pool(name="pstr", bufs=2, space="PSUM") as pst,
            tc.tile_pool(name="psrow", bufs=2, space="PSUM") as psr,
            tc.tile_pool(name="dram", bufs=1, space="DRAM") as dr,  # noqa
        ):
            # ---------- constants ----------
            nonce_t = cst.tile([128, 128], F32)
            nc.sync.dma_start(out=nonce_t[:, 0:nonce_k], in_=nonce_d[:])
            nz_t = cst.tile([128, 1], F32)
            nc.vector.tensor_reduce(nz_t[:], nonce_t[:, 0:nonce_k], axis=AX.X, op=OP.add)
            idx_t = cst.tile([128, NB], U32)
            nc.sync.dma_start(out=idx_t[:], in_=idx_d[:])
            sel_t = cst.tile([128, 1], F32)
            nc.sync.dma_start(out=sel_t[:], in_=sel_d[:])
            ut32 = None
            if cfg["bd"] == "f32":
                ut32 = cst.tile([128, 128], F32, name="ut32")
                nc.sync.dma_start(out=ut32[:], in_=ut32_d[:])
            ut_r = cst.tile([128, 128], F32R)
            nc.sync.dma_start(out=ut_r[:], in_=utr_d[:])
            utms = cst.tile([128, 128], F32R)
            nc.sync.dma_start(out=utms[:], in_=utms_d[:])
            eneg = cst.tile([128, 128], F32R)
            nc.sync.dma_start(out=eneg[:], in_=eneg_d[:])
            idm_r = cst.tile([128, 128], F32R)
            nc.sync.dma_start(out=idm_r[:], in_=idm_r_d[:])
            idm32 = None
            if OUT32:
                idm32 = cst.tile([128, 128], F32, name="idm32")
                nc.sync.dma_start(out=idm32[:], in_=idm32_d[:])
            or32 = None
            if cfg["bd"] == "f32":
                or32 = cst.tile([1, 128], F32, name="or32")
                nc.sync.dma_start(out=or32[:], in_=or32_d[:])
            orr = cst.tile([1, 128], F32R)
            nc.sync.dma_start(out=orr[:], in_=orr_d[:])
            oc32 = None
            if cfg["bd"] == "f32":
                oc32 = cst.tile([128, 1], F32, name="oc32")
                nc.sync.dma_start(out=oc32[:], in_=oc32_d[:])
            ocr = cst.tile([128, 1], F32R)
            nc.sync.dma_start(out=ocr[:], in_=ocr_d[:])
            embT = cst.tile([128, ND, VOCAB], ODT)
            nc.sync.dma_start(out=embT[:], in_=embT_d[:].rearrange("(a p) v -> p a v", p=128))
            dcyd = [cst.tile([128, 128], CDT, tag=f"dcyd{l}", name=f"dcyd{l}") for l in range(L)]
            dcys = [cst.tile([128, 128], CDT, tag=f"dcys{l}", name=f"dcys{l}") for l in range(L)]
            for l in range(L):
                nc.sync.dma_start(out=dcyd[l][:], in_=dcyd_d[l])
                nc.sync.dma_start(out=dcys[l][:], in_=dcys_d[l])
            eps_t = cst.tile([128, 1], F32)
            nc.vector.memset(eps_t[:], 1e-3)

            # ---------- state ----------
            h = st.tile([128, NB, D], F32)       # row layout, partition=time%128
            h_r = st.tile([128, NB, D], F32R)
            xTc = [st.tile([128, ND, CHUNK], F32R, tag=f"xTc{i}", name=f"xTc{i}")
                   for i in range(NCH)]
            magsq = st.tile([128, NB], F32)
            mag = st.tile([128, NB], F32)
            

            # ---------- layer-0 weight prefetch ----------
            def load_wts(l):
                w1c, w2c = [], []
                for hb in range(NHB):
                    t1 = wp.tile([128, ND, 128], MLPDT, tag="w1c", name="w1c", bufs=cfg["w1_bufs"])
                    nc.sync.dma_start(
                        out=t1[:],
                        in_=w1_d[l, :, hb * 128:(hb + 1) * 128].rearrange(
                            "(a p) c -> p a c", p=128))
                    w1c.append(t1)
                    t2 = wp.tile([128, D], MLPDT, tag="w2c", name="w2c", bufs=cfg["w2_bufs"])
                    nc.sync.dma_start(out=t2[:], in_=w2_d[l, hb * 128:(hb + 1) * 128, :])
                    w2c.append(t2)
                b1sb = wp.tile([128, NHB], F32, tag="b1sb", bufs=2)
                nc.sync.dma_start(out=b1sb[:], in_=b1_d[l])
                b2row = wp.tile([1, D], MLPDT, tag="b2row", bufs=2)
                nc.sync.dma_start(out=b2row[:], in_=b2_d[l])
                return w1c, w2c, b1sb, b2row


            import contextlib
            loop_cm = tc.For_i(0, cfg["hw_loop"], 1) if cfg["hw_loop"] > 1 else contextlib.nullcontext()
            with loop_cm:
              for rep in range(cfg["repeat"]):
                # ---------- embedding gather ----------
                for k in range(NB):
                    nc.gpsimd.indirect_dma_start(
                        out=h[:, k, :], out_offset=None,
                        in_=emb_d[:],
                        in_offset=IndirectOffsetOnAxis(ap=idx_t[:, k:k + 1], axis=0))

                # ---------- phase A: tanh(cumprod) colsums -> cc_tc ----------
                BD32 = cfg["bd"] == "f32"
                utb = ut32 if BD32 else ut_r
                orb = or32 if BD32 else orr
                ocb = oc32 if BD32 else ocr
                BDT = F32 if BD32 else F32R
                def tc_block(k, dst):
                    cpw = wk.tile([128, D], F32, tag="scr", name="cpw")
                    nc.vector.tensor_tensor_scan(cpw[:], h[:, k, :], h[:, k, :], 1.0,
                                                 OP.mult, OP.bypass)
                    nc.scalar.activation(dst[:], cpw[:], ACT.Tanh)

                msum = None
                for k in range(NB):
                    tcw = wk.tile([128, D], BDT, tag="tcw", name="tcw")
                    tc_block(k, tcw)
                    cs = psr.tile([1, D], F32, tag="row")
                    nc.tensor.matmul(cs[:], ocb[:], tcw[:], start=True, stop=True)
                    nm = rw.tile([1, D], F32, tag="msump", name="msump")
                    if k == 0:
                        nc.vector.tensor_copy(nm[:], cs[:])
                    else:
                        nc.vector.tensor_tensor(nm[:], msum[:], cs[:], op=OP.add)
                    msum = nm
                nc.sync.dma_start(out=cc_tc_in[:], in_=msum[:])  # noqa: phaseA
                if not cfg["no_cc"]:
                    nc.gpsimd.collective_compute("AllGather", OP.bypass, replica_groups=GROUPS,
                                                 ins=[cc_tc_in[:]], outs=[cc_tc_out[:]])
                base_raw = rw.tile([1, D], F32, tag="braw", bufs=1, name="base_raw")
                nc.sync.dma_start(out=base_raw[:], in_=cc_tc_out[0:1, :])

                # ---------- phase B: bd = cumsum(tc) + base; h = h*(1+bd) ----------
                off = rw.tile([1, D], BDT, tag="offp", name="offp")
                nc.vector.tensor_scalar(off[:], base_raw[:], sel_t[0:1, :], None, OP.mult)
                for k in range(NB):
                    tcw = wk.tile([128, D], BDT, tag="tcw", name="tcw")
                    tc_block(k, tcw)
                    bd_ps = psm.tile([128, D], F32, tag="mm")
                    nc.tensor.matmul(bd_ps[:], utb[:], tcw[:], start=True, stop=False)
                    nc.tensor.matmul(bd_ps[:], orb[:], off[:], start=False, stop=True)
                    if k + 1 < NB:
                        cs = psr.tile([1, D], F32, tag="row")
                        nc.tensor.matmul(cs[:], ocb[:], tcw[:], start=True, stop=True)
                        noff = rw.tile([1, D], BDT, tag="offp", name="offp")
                        nc.vector.tensor_tensor(noff[:], off[:], cs[:], op=OP.add)
                        off = noff
                    htmp = wk.tile([128, D], F32, tag="htmp")
                    nc.vector.scalar_tensor_tensor(htmp[:], bd_ps[:], 1.0, h[:, k, :],
                                                   op0=OP.add, op1=OP.mult)
                    nc.vector.tensor_copy(h[:, k, :], htmp[:])
                    nc.vector.tensor_copy(h_r[:, k, :], htmp[:])

                # ---------- layers ----------
                for l in range(L):
                    w1c, w2c, b1sb, b2row = load_wts(l)

                    # -- cc prep: colsums of h_r + my_sum --
                    msum = None
                    for k in range(NB):
                        cs = psr.tile([1, D], F32, tag="row")
                        nc.tensor.matmul(cs[:], ocr[:], h_r[:, k, :], start=True, stop=True)
                        nm = rw.tile([1, D], F32, tag="msump", name="msump")
                        if k == 0:
                            nc.vector.tensor_copy(nm[:], cs[:])
                        else:
                            nc.vector.tensor_tensor(nm[:], msum[:], cs[:], op=OP.add)
                        msum = nm
                    nc.sync.dma_start(out=cc_in[l][0:128, :], in_=h[:, NB - 1, :])
                    nc.sync.dma_start(out=cc_in[l][128:129, :], in_=msum[:])
                    if not cfg["no_cc"]:
                        nc.gpsimd.collective_compute("AllGather", OP.bypass, replica_groups=GROUPS,
                                                     ins=[cc_in[l][:]], outs=[cc_out[l][:]])
                    base_raw = rw.tile([1, D], F32, tag="braw", bufs=1, name="base_raw")
                    nc.sync.dma_start(out=base_raw[:], in_=cc_out[l][128:129, :])
                    halo_raw = wk.tile([128, D], F32, tag="htmp", name="halo_raw")
                    nc.sync.dma_start(out=halo_raw[:], in_=cc_out[l][0:128, :])
                    halo32 = None
                    if CTX32:
                        halo32 = st.tile([128, D], F32, tag="halo32", name="halo32")
                        nc.vector.tensor_scalar(halo32[:], halo_raw[:], sel_t[:], None, OP.mult)
                        halo_r = st.tile([128, D], F32R, tag="halo_r", name="halo_r")
                        nc.vector.tensor_copy(halo_r[:], halo32[:])
                    else:
                        halo_r = st.tile([128, D], F32R, tag="halo_r", name="halo_r")
                        nc.vector.tensor_scalar(halo_r[:], halo_raw[:], sel_t[:], None, OP.mult)

                    # -- diff pass (fp32r): diff = hcum - h_prev; magsq --
                    offr = rw.tile([1, D], F32R, tag="offp", name="offrp")
                    nc.vector.tensor_scalar(offr[:], base_raw[:], sel_t[0:1, :], None, OP.mult)
                    for k in range(NB):
                        df = psm.tile([128, D], F32, tag="mm")
                        nc.tensor.matmul(df[:], utms[:], h_r[:, k, :], start=True, stop=False)
                        nc.tensor.matmul(df[:], orr[:], offr[:], start=False, stop=False)
                        prev_r = halo_r[:] if k == 0 else h_r[:, k - 1, :]
                        nc.tensor.matmul(df[:], eneg[:], prev_r, start=False, stop=True)
                        if k + 1 < NB:
                            cs = psr.tile([1, D], F32, tag="row")
                            nc.tensor.matmul(cs[:], ocr[:], h_r[:, k, :], start=True, stop=True)
                            noffr = rw.tile([1, D], F32R, tag="offp", name="offrp")
                            nc.vector.tensor_tensor(noffr[:], offr[:], cs[:], op=OP.add)
                            offr = noffr
                        scr = wk.tile([128, D], F32, tag="scr")
                        nc.scalar.activation(scr[:], df[:], ACT.Square,
                                             accum_out=magsq[:, k:k + 1])
                    nc.scalar.activation(mag[:], magsq[:], ACT.Sqrt)

                    # -- ctx + LN + transpose per block --
                    for k in range(NB):
                        cx = psm.tile([128, D], F32, tag="mm")
                        hsrc = h if CTX32 else h_r
                        prev = (halo32 if CTX32 else halo_r)[:] if k == 0 else hsrc[:, k - 1, :]
                        nc.tensor.matmul(cx[:], dcys[l][:], prev, start=True, stop=False)
                        nc.tensor.matmul(cx[:], dcyd[l][:], hsrc[:, k, :], start=False, stop=True)
                        ctx_sb = wk.tile([128, D], F32, tag="ctx_sb")
                        nc.scalar.activation(ctx_sb[:], cx[:], ACT.Abs, scale=mag[:, k:k + 1])
                        # LN
                        red = rw.tile([128, 1], F32, tag="red")
                        scr = wk.tile([128, D], F32, tag="scr")
                        sq = rw.tile([128, 1], F32, tag="sq")
                        nc.vector.scalar_tensor_tensor(scr[:], ctx_sb[:], 0.0, ctx_sb[:],
                                                       op0=OP.add, op1=OP.mult,
                                                       accum_out=sq[:])
                        nc.vector.tensor_reduce(red[:], ctx_sb[:], axis=AX.X, op=OP.add)
                        mu = rw.tile([128, 1], F32, tag="mu")
                        nc.vector.tensor_scalar(mu[:], red[:], 1.0 / D, None, OP.mult)
                        musq = rw.tile([128, 1], F32, tag="musq")
                        nc.vector.scalar_tensor_tensor(musq[:], mu[:], 0.0, mu[:],
                                                       op0=OP.add, op1=OP.mult)
                        vs = rw.tile([128, 1], F32, tag="vs")
                        nc.vector.scalar_tensor_tensor(vs[:], sq[:], 1.0 / D, musq[:],
                                                       op0=OP.mult, op1=OP.subtract)
                        sd = rw.tile([128, 1], F32, tag="sd")
                        nc.scalar.activation(sd[:], vs[:], ACT.Sqrt, bias=eps_t[:])
                        rs = rw.tile([128, 1], F32, tag="rs")
                        nc.vector.reciprocal(rs[:], sd[:])
                        xhat = wk.tile([128, D], F32R, tag="xhat")
                        nc.vector.tensor_scalar(xhat[:], ctx_sb[:], mu[:], rs[:],
                                                OP.subtract, OP.mult)
                        for j in range(ND):
                            tr = pst.tile([128, 128], F32R, tag="tr")
                            nc.tensor.transpose(tr[:], xhat[:, j * 128:(j + 1) * 128], idm_r[:])
                            co, cj = k // RB_PER_CH, (k % RB_PER_CH) * 128
                            nc.vector.tensor_copy(xTc[co][:, j, cj:cj + 128], tr[:])

                    # -- MLP --
                    for c in range(NCH):
                        gt = st.tile([128, NHB, CHUNK], F32R, tag="gt", bufs=cfg.get("gt_bufs", 1))
                        for hb in range(NHB):
                            g_ps = psm.tile([128, CHUNK], F32, tag="mm")
                            for db in range(ND):
                                nc.tensor.matmul(g_ps[:], w1c[hb][:, db, :],
                                                 xTc[c][:, db, :],
                                                 start=(db == 0), stop=(db == ND - 1))
                            nc.scalar.activation(gt[:, hb, :], g_ps[:], ACT.Gelu,
                                                 bias=b1sb[:, hb:hb + 1])
                        for rb in range(RB_PER_CH):
                            k = c * RB_PER_CH + rb
                            y_ps = psm.tile([128, D], F32, tag="mm")
                            for hb in range(NHB):
                                nc.tensor.matmul(y_ps[:], gt[:, hb, rb * 128:(rb + 1) * 128],
                                                 w2c[hb][:], start=(hb == 0), stop=False)
                            nc.tensor.matmul(y_ps[:], orr[:], b2row[:], start=False, stop=True)
                            htmp = wk.tile([128, D], F32, tag="htmp")
                            nc.vector.tensor_tensor(htmp[:], y_ps[:], h[:, k, :], op=OP.add)
                            nc.vector.tensor_copy(h[:, k, :], htmp[:])
                            nc.vector.tensor_copy(h_r[:, k, :], htmp[:])

                # ---------- output projection ----------
                idm = idm32 if OUT32 else idm_r
                hs = h if OUT32 else h_r
                hT = st.tile([128, ND, R], F32 if OUT32 else F32R, name="hT")
                def hT_ap(j, sl):
                    return hT[:, j, sl]
                for k in range(NB):
                    for j in range(ND):
                        tr = pst.tile([128, 128], F32 if OUT32 else F32R, tag="tr", name="tr")
                        nc.tensor.transpose(tr[:], hs[:, k, j * 128:(j + 1) * 128], idm[:])
                        nc.vector.tensor_copy(hT_ap(j, slice(k * 128, (k + 1) * 128)), tr[:])
                for vb in range(VOCAB // 128):
                    for rc in range(R // 512):
                        op_ps = psm.tile([128, 512], F32, tag="mm")
                        for db in range(ND):
                            nc.tensor.matmul(op_ps[:], embT[:, db, vb * 128:(vb + 1) * 128],
                                             hT_ap(db, slice(rc * 512, (rc + 1) * 512)),
                                             start=(db == 0), stop=(db == ND - 1))
                        osb = wk.tile([128, 512], F32, tag="scr", name="osb")
                        nc.vector.tensor_copy(osb[:], op_ps[:])
                        nc.sync.dma_start(out=out_d[vb * 128:(vb + 1) * 128,
                                                    rc * 512:(rc + 1) * 512],
                                          in_=osb[:])


    nc.compile()
    return nc, cfg


def host_prep(inputs, cfg=None):
    cfg = {**DEFAULT_CFG, **(cfg or {})}
    import hashlib, json
    nonce_k = int(hashlib.sha256(json.dumps(cfg, sort_keys=True).encode()).hexdigest(), 16) % 97 + 2
    x = np.asarray(inputs["x"]).astype(np.uint32)
    embed = np.asarray(inputs["embed"], np.float32)
    k = np.asarray(inputs["k"], np.float32)
    g = np.asarray(inputs["ln_gamma"], np.float32)
    be = np.asarray(inputs["ln_beta"], np.float32)
    W1 = np.asarray(inputs["W1"], np.float32)
    b1 = np.asarray(inputs["b1"], np.float32)
    W2 = np.asarray(inputs["W2"], np.float32)
    b2 = np.asarray(inputs["b2"], np.float32)

    mlp_rnd = rne12 if cfg["mlp"] == "f32r" else (lambda a: a)
    ctx_rnd = rne12 if cfg["ctx"] == "f32r" else (lambda a: a)
    out_rnd = rne12 if cfg["out"] == "f32r" else (lambda a: a)

    W1f = np.ascontiguousarray(mlp_rnd(g[:, :, None] * W1))
    b1f = np.einsum("ld,ldh->lh", be, W1) + b1
    b1f = np.ascontiguousarray(b1f.reshape(L, NHB, 128).transpose(0, 2, 1))
    W2r = np.ascontiguousarray(mlp_rnd(W2))
    b2r = np.ascontiguousarray(mlp_rnd(b2)).reshape(L, 1, D)

    lam = np.exp(-softplus64(k))  # (L,) f64
    assert (lam ** 128 < 1e-40).all(), f"decay band >128 unsupported: lam={lam}"
    i_ = np.arange(128)
    dcyd = np.zeros((L, 128, 128), np.float64)
    dcys = np.zeros((L, 128, 128), np.float64)
    for l in range(L):
        e = i_[None, :] - i_[:, None]          # i - j indexed [j, i]
        dcyd[l] = np.where(e >= 0, lam[l] ** np.maximum(e, 0), 0.0)
        dcys[l] = lam[l] ** (e + 128.0)
    dcyd = ctx_rnd(dcyd.astype(np.float32))
    dcys = ctx_rnd(dcys.astype(np.float32))

    ut32 = np.triu(np.ones((128, 128), np.float32))
    utr = ut32.copy()
    utms = np.triu(np.ones((128, 128), np.float32)).copy()
    utms[i_[:-1], i_[:-1] + 1] = 0.0           # M^T[j,t]: 1 if j<=t and j != t-1
    eneg = np.zeros((128, 128), np.float32)
    eneg[127, 0] = -1.0
    idm = np.eye(128, dtype=np.float32)
    embT = np.ascontiguousarray(out_rnd(embed.T))

    shared = dict(nonce=np.zeros((128, nonce_k), np.float32),
                  emb=embed, w1=W1f, w2=W2r, b1=b1f, b2=b2r,
                  ut32=ut32, utr=utr, utms=utms, eneg=eneg, dcyd=dcyd, dcys=dcys,
                  embT=embT, idm_r=idm, idm32=idm,
                  or32=np.ones((1, 128), np.float32),
                  orr=np.ones((1, 128), np.float32),
                  oc32=np.ones((128, 1), np.float32),
                  ocr=np.ones((128, 1), np.float32))
    in_maps = []
    for c in range(8):
        b, hf = c // 2, c % 2
        xi = x[b, hf * R:(hf + 1) * R]
        idx = np.ascontiguousarray(xi.reshape(NB, 128).T)  # [p, k]
        m = dict(shared)
        m["idx"] = idx
        m["sel"] = np.full((128, 1), float(hf), np.float32)
        in_maps.append(m)
    return in_maps


def assemble(results):
    out = np.zeros((B, T, VOCAB), np.float32)
    for c in range(8):
        b, hf = c // 2, c % 2
        out[b, hf * R:(hf + 1) * R, :] = results[c]["logitsT"].T
    return out


# ---------------- PJRT runner ----------------
import numpy as np
import jax
from jax.sharding import Mesh, PartitionSpec
from jax.experimental.shard_map import shard_map
import concourse.mybir as mybir
from concourse import bass2jax
from concourse.bass2jax import _bass_exec_p, install_neuronx_cc_hook


def make_runner(nc, n_cores):
    install_neuronx_cc_hook()
    partition_name = nc.partition_id_tensor.name if nc.partition_id_tensor else None
    in_names, out_names, out_avals, zero_outs = [], [], [], []
    for alloc in nc.m.functions[0].allocations:
        if not isinstance(alloc, mybir.MemoryLocationSet):
            continue
        name = alloc.memorylocations[0].name
        if alloc.kind == "ExternalInput":
            if name != partition_name:
                in_names.append(name)
        elif alloc.kind == "ExternalOutput":
            shape = tuple(alloc.tensor_shape)
            dtype = mybir.dt.np(alloc.dtype)
            out_names.append(name)
            out_avals.append(jax.core.ShapedArray(shape, dtype))
            zero_outs.append(np.zeros(shape, dtype))
    n_params = len(in_names)
    n_outs = len(out_avals)
    all_in_names = in_names + out_names + ([partition_name] if partition_name else [])

    def _body(*args):
        operands = list(args)
        if partition_name is not None:
            operands.append(bass2jax.partition_id_tensor())
        outs = _bass_exec_p.bind(
            *operands,
            out_avals=tuple(out_avals),
            in_names=tuple(all_in_names),
            out_names=tuple(out_names),
            lowering_input_output_aliases=(),
            sim_require_finite=True,
            sim_require_nnan=True,
            nc=nc,
        )
        return tuple(outs)

    devices = jax.devices()[:n_cores]
    mesh = Mesh(np.asarray(devices), ("core",))
    in_specs = (PartitionSpec("core"),) * (n_params + n_outs)
    out_specs = (PartitionSpec("core"),) * len(out_names)
    # NOTE: no donation so the callable can be invoked repeatedly.
    sharded = jax.jit(
        shard_map(_body, mesh=mesh, in_specs=in_specs, out_specs=out_specs,
                  check_rep=False),
        keep_unused=True,
    )

    from jax.sharding import NamedSharding
    sh = NamedSharding(mesh, PartitionSpec("core"))

    def prepare(in_maps):
        per_core = [[np.asarray(m[name]) for name in in_names] for m in in_maps]
        concat_in = [np.concatenate([per_core[c][i] for c in range(n_cores)], axis=0)
                     for i in range(n_params)]
        concat_zeros = [np.zeros((n_cores * z.shape[0], *z.shape[1:]), z.dtype)
                        for z in zero_outs]
        dev = [jax.device_put(a, sh) for a in concat_in + concat_zeros]
        jax.block_until_ready(dev)
        return dev

    def run_dev(dev):
        out_arrs = sharded(*dev)
        jax.block_until_ready(out_arrs)
        return out_arrs

    def unpack(out_arrs):
        return [
            {name: np.asarray(out_arrs[i]).reshape(n_cores, *out_avals[i].shape)[c]
             for i, name in enumerate(out_names)}
            for c in range(n_cores)
        ]

    def run(in_maps):
        return unpack(run_dev(prepare(in_maps)))

    run.prepare = prepare
    run.run_dev = run_dev
    run.unpack = unpack
    return run


# ---------------- public entry point ----------------
_CACHE = {}


def kernel(**inputs):
    if "k" not in _CACHE:
        nc, cfg = build()
        _CACHE["k"] = (nc, cfg, make_runner(nc, 8))
    nc, cfg, run = _CACHE["k"]
    in_maps = host_prep(inputs, cfg)
    results = run(in_maps)
    return assemble(results)

